# revision 9
# baseline (speedup 1.0000x reference)
"""Multi-head causal attention (B=4, T=2048, C=1024, H=16) on 8 trn2 NeuronCores.

Sharding: core i handles batch b = i//2 and 8 of the 16 heads (half = i%2).
Per core (all in fp32r on the TensorEngine, full rate at N>=256):
  phase 1: QKV projection for its 8 heads   (x_b @ [wq;wk;wv]_shard^T)
  phase 2: causal attention, scores kept transposed (S^T[tk, tq]) so no PE
           transposes are needed; softmax denominator folded into the AV
           matmul via a ones-column appended to V
  phase 3: partial output projection over its 512 channels -> y^T partial
Host: y[b] = (yT_core(2b) + yT_core(2b+1)).T + b_proj  (head-half reduce).

No collectives; the program is SPMD-uniform, only input data differs per core.
All operand transposes (x^T, w^T) are done host-side in numpy and passed as
extra input tensors.
"""
import sys

sys.path.insert(0, "/opt/trn_rl_repo")

import numpy as np
from contextlib import ExitStack

import concourse.bass as bass
import concourse.tile as tile
from concourse import bacc, mybir
from concourse.bass_utils import run_bass_kernel_spmd

B, T, C = 4, 2048, 1024
H = 16
HD = 64          # head dim
NHL = 8          # heads per core
CL = NHL * HD    # local channels = 512
NCC = C // 128   # cin chunks = 8
NTB = T // 128   # token blocks = 16
NPAN = T // 512  # token panels = 4
NQG = T // 512   # query groups of 512 = 4
NHP = NHL // 2   # head pairs = 4

F32 = mybir.dt.float32
F32R = mybir.dt.float32r


def _build_program():
    nc = bacc.Bacc("TRN2", target_bir_lowering=False, debug=False, num_devices=8)

    xT = nc.dram_tensor("xT", [C, T], F32R, kind="ExternalInput").ap()
    wqkT = nc.dram_tensor("wqkT", [C, 2 * CL], F32R, kind="ExternalInput").ap()
    wvT = nc.dram_tensor("wvT", [C, CL], F32R, kind="ExternalInput").ap()
    wprojT = nc.dram_tensor("wprojT", [CL, C], F32R, kind="ExternalInput").ap()
    masks = nc.dram_tensor("masks", [128, 4 * 512], F32R, kind="ExternalInput").ap()
    ones = nc.dram_tensor("ones", [128, NTB * NHL], F32R, kind="ExternalInput").ap()
    yT = nc.dram_tensor("yT", [C, T], F32, kind="ExternalOutput").ap()

    with tile.TileContext(nc) as tc, ExitStack() as ctx:
        persist = ctx.enter_context(tc.tile_pool(name="persist", bufs=1))
        # Q^T/K^T per head pair: [128, T], partitions = 2 heads x 64
        QT = [persist.tile([128, T], F32R, tag=f"qt{p}", name=f"qt{p}") for p in range(NHP)]
        KT = [persist.tile([128, T], F32R, tag=f"kt{p}", name=f"kt{p}") for p in range(NHP)]
        # V with a ones column appended per head: [tk mod 128, tb, head, 65]
        Vg = persist.tile([128, NTB, NHL, HD + 1], F32R, tag="vg")

        # ones column of V_aug (softmax denominator trick), via strided DMA
        nc.sync.dma_start(
            out=Vg.rearrange("p tb h d -> p (tb h) d")[:, :, HD : HD + 1],
            in_=ones.rearrange("p (n o) -> p n o", o=1),
        )

        # ---------------- phase 1: QKV projection ----------------
        with ExitStack() as p1:
            wpool = p1.enter_context(tc.tile_pool(name="p1w", bufs=1))
            xpool = p1.enter_context(tc.tile_pool(name="p1x", bufs=2))
            ps1 = p1.enter_context(tc.tile_pool(name="p1ps", bufs=4, space="PSUM"))

            wqk_sb = wpool.tile([128, NCC, 2 * CL], F32R, tag="wqk")
            wv_sb = wpool.tile([128, NCC, CL], F32R, tag="wv")
            nc.sync.dma_start(out=wqk_sb, in_=wqkT.rearrange("(cc p) n -> p cc n", p=128))
            nc.sync.dma_start(out=wv_sb, in_=wvT.rearrange("(cc p) n -> p cc n", p=128))

            xTr = xT.rearrange("(cc p) t -> p cc t", p=128)
            for pan in range(NPAN):
                xp = xpool.tile([128, NCC, 512], F32R, tag="xp")
                nc.sync.dma_start(out=xp, in_=xTr[:, :, pan * 512 : (pan + 1) * 512])

                # Q (chunks 0-3) and K (chunks 4-7): out [chan 128, tok 512]
                for ch in range(8):
                    ps = ps1.tile([128, 512], F32, tag="ps")
                    for cc in range(NCC):
                        nc.tensor.matmul(
                            ps,
                            wqk_sb[:, cc, ch * 128 : (ch + 1) * 128],
                            xp[:, cc, :],
                            start=(cc == 0),
                            stop=(cc == NCC - 1),
                        )
                    dest = QT[ch] if ch < 4 else KT[ch - 4]
                    nc.vector.tensor_copy(
                        dest[:, pan * 512 : (pan + 1) * 512], ps
                    )
                # V: out [tok 128, chan 512] for the 4 token blocks of the panel
                for tbl in range(4):
                    tb = pan * 4 + tbl
                    ps = ps1.tile([128, 512], F32, tag="ps")
                    for cc in range(NCC):
                        nc.tensor.matmul(
                            ps,
                            xp[:, cc, tbl * 128 : (tbl + 1) * 128],
                            wv_sb[:, cc, :],
                            start=(cc == 0),
                            stop=(cc == NCC - 1),
                        )
                    nc.vector.tensor_copy(
                        Vg[:, tb, :, 0:HD],
                        ps.rearrange("p (h d) -> p h d", h=NHL),
                    )

        # ---------------- phases 2+3 ----------------
        with ExitStack() as p23:
            otpool = p23.enter_context(tc.tile_pool(name="ot", bufs=1))
            OT = [otpool.tile([128, T], F32R, tag=f"ot{p}", name=f"ot{p}") for p in range(NHP)]
            masks_sb = otpool.tile([128, 4 * 512], F32R, tag="masks")
            nc.sync.dma_start(out=masks_sb, in_=masks)

            # ---- phase 2: attention ----
            with ExitStack() as p2:
                ppool = p2.enter_context(tc.tile_pool(name="p2p", bufs=6))
                psS = p2.enter_context(tc.tile_pool(name="p2s", bufs=4, space="PSUM"))
                psO = p2.enter_context(tc.tile_pool(name="p2o", bufs=2, space="PSUM"))

                for hp in range(NHP):
                    for qg in range(NQG):
                        nkb = 4 * qg + 4  # causal extent in k-blocks
                        OA = psO.tile([HD + 1, 512], F32, tag="o")
                        OB = psO.tile([HD + 1, 512], F32, tag="o")
                        qsl = slice(qg * 512, (qg + 1) * 512)
                        for jk in range(nkb):
                            ksl = slice(jk * 128, (jk + 1) * 128)
                            SA = psS.tile([128, 512], F32, tag="s")
                            SB = psS.tile([128, 512], F32, tag="s")
                            nc.tensor.matmul(
                                SA, KT[hp][0:64, ksl], QT[hp][0:64, qsl],
                                start=True, stop=True,
                            )
                            nc.tensor.matmul(
                                SB, KT[hp][64:128, ksl], QT[hp][64:128, qsl],
                                start=True, stop=True,
                            )
                            PA = ppool.tile([128, 512], F32R, tag="pt")
                            PB = ppool.tile([128, 512], F32R, tag="pt")
                            nc.scalar.activation(PA, SA, mybir.ActivationFunctionType.Exp)
                            nc.scalar.activation(PB, SB, mybir.ActivationFunctionType.Exp)
                            if jk >= 4 * qg:  # diagonal-intersecting block
                                r = jk - 4 * qg
                                msl = slice(r * 512, (r + 1) * 512)
                                nc.vector.tensor_mul(PA, PA, masks_sb[:, msl])
                                nc.vector.tensor_mul(PB, PB, masks_sb[:, msl])
                            nc.tensor.matmul(
                                OA, Vg[:, jk, 2 * hp, :], PA,
                                start=(jk == 0), stop=(jk == nkb - 1),
                            )
                            nc.tensor.matmul(
                                OB, Vg[:, jk, 2 * hp + 1, :], PB,
                                start=(jk == 0), stop=(jk == nkb - 1),
                            )
                        # normalize: 1/denominator, broadcast over 64 partitions,
                        # multiply while copying O^T out of PSUM
                        for h01, Oacc in ((0, OA), (1, OB)):
                            drow = ppool.tile([1, 512], F32, tag="drow", bufs=2)
                            nc.vector.tensor_copy(drow, Oacc[HD : HD + 1, :])
                            rcp = ppool.tile([1, 512], F32, tag="rcp", bufs=2)
                            nc.vector.reciprocal_approx_fast(rcp, drow)
                            bc = ppool.tile([64, 512], F32, tag="bc", bufs=2)
                            nc.gpsimd.partition_broadcast(bc, rcp)
                            nc.vector.tensor_mul(
                                OT[hp][h01 * 64 : (h01 + 1) * 64, qsl],
                                Oacc[0:HD, :],
                                bc,
                            )

            # ---- phase 3: output projection (partial, local channels) ----
            with ExitStack() as p3:
                w3pool = p3.enter_context(tc.tile_pool(name="p3w", bufs=1))
                o3pool = p3.enter_context(tc.tile_pool(name="p3o", bufs=4))
                ps3 = p3.enter_context(tc.tile_pool(name="p3ps", bufs=4, space="PSUM"))

                wp_sb = w3pool.tile([128, NHP, C], F32R, tag="wp")
                nc.sync.dma_start(
                    out=wp_sb, in_=wprojT.rearrange("(cc p) n -> p cc n", p=128)
                )
                for ch in range(C // 128):
                    for tg in range(NPAN):
                        ps = ps3.tile([128, 512], F32, tag="ps")
                        for cc in range(NHP):
                            nc.tensor.matmul(
                                ps,
                                wp_sb[:, cc, ch * 128 : (ch + 1) * 128],
                                OT[cc][:, tg * 512 : (tg + 1) * 512],
                                start=(cc == 0),
                                stop=(cc == NHP - 1),
                            )
                        o = o3pool.tile([128, 512], F32, tag="o3")
                        nc.vector.tensor_copy(o, ps)
                        nc.sync.dma_start(
                            out=yT[ch * 128 : (ch + 1) * 128, tg * 512 : (tg + 1) * 512],
                            in_=o,
                        )

    nc.compile()
    return nc


_NC = None


def _get_program():
    global _NC
    if _NC is None:
        _NC = _build_program()
    return _NC


def _make_masks() -> np.ndarray:
    # mask_r[tk, tq] = 1 if tq >= 128*r + tk else 0, r = 0..3, laid side by side
    tk = np.arange(128)[:, None]
    tq = np.arange(512)[None, :]
    return np.concatenate(
        [(tq >= 128 * r + tk).astype(np.float32) for r in range(4)], axis=1
    )


def make_in_maps(x, w_qkv, w_proj):
    x = np.asarray(x, dtype=np.float32)
    w_qkv = np.asarray(w_qkv, dtype=np.float32)
    w_proj = np.asarray(w_proj, dtype=np.float32)
    masks = _make_masks()
    scale = np.float32(HD ** -0.5)
    in_maps = []
    for i in range(8):
        b, hh = i // 2, i % 2
        ch0 = hh * CL
        wq = w_qkv[ch0 : ch0 + CL, :] * scale
        wk = w_qkv[C + ch0 : C + ch0 + CL, :]
        wv = w_qkv[2 * C + ch0 : 2 * C + ch0 + CL, :]
        in_maps.append(
            {
                "xT": np.ascontiguousarray(x[b].T),
                "wqkT": np.ascontiguousarray(np.concatenate([wq, wk], 0).T),
                "wvT": np.ascontiguousarray(wv.T),
                "wprojT": np.ascontiguousarray(w_proj[:, ch0 : ch0 + CL].T),
                "masks": masks,
                "ones": np.ones((128, NTB * NHL), dtype=np.float32),
            }
        )
    return in_maps


def assemble_output(results, b_proj):
    b_proj = np.asarray(b_proj, dtype=np.float32)
    y = np.empty((B, T, C), dtype=np.float32)
    for b in range(B):
        y[b] = (results[2 * b]["yT"] + results[2 * b + 1]["yT"]).T + b_proj
    return y


def kernel(x, w_qkv, w_proj, b_proj):
    nc = _get_program()
    in_maps = make_in_maps(x, w_qkv, w_proj)
    res = run_bass_kernel_spmd(nc, in_maps, core_ids=list(range(8)))
    return assemble_output(res.results, b_proj)


if __name__ == "__main__":
    rng = np.random.default_rng(0)
    x = rng.standard_normal((B, T, C), dtype=np.float32)
    w_qkv = (rng.standard_normal((3 * C, C), dtype=np.float32) * C ** -0.5).astype(
        np.float32
    )
    w_proj = (rng.standard_normal((C, C), dtype=np.float32) * C ** -0.5).astype(
        np.float32
    )
    b_proj = np.zeros(C, dtype=np.float32)
    y = kernel(x, w_qkv, w_proj, b_proj)
    print("out", y.shape, y.dtype, float(np.abs(y).max()))


# revision 10
# speedup vs baseline: 58.6027x; 58.6027x over previous
"""Multi-head causal attention (B=4, T=2048, C=1024, H=16) on 8 trn2 NeuronCores.

Sharding: core i handles batch b = i//2 and 8 of the 16 heads (half = i%2).
Per core (all in fp32r on the TensorEngine, full rate at N>=256):
  phase 1: QKV projection for its 8 heads   (x_b @ [wq;wk;wv]_shard^T)
  phase 2: causal attention, scores kept transposed (S^T[tk, tq]) so no PE
           transposes are needed; softmax denominator folded into the AV
           matmul via a ones-column appended to V
  phase 3: partial output projection over its 512 channels -> y^T partial
Host: y[b] = (yT_core(2b) + yT_core(2b+1)).T + b_proj  (head-half reduce).

No collectives; the program is SPMD-uniform, only input data differs per core.
All operand transposes (x^T, w^T) are done host-side in numpy and passed as
extra input tensors.
"""
import sys

sys.path.insert(0, "/opt/trn_rl_repo")

import numpy as np
from contextlib import ExitStack

import concourse.bass as bass
import concourse.tile as tile
from concourse import bacc, mybir
from concourse.bass_utils import run_bass_kernel_spmd

B, T, C = 4, 2048, 1024
H = 16
HD = 64          # head dim
NHL = 8          # heads per core
CL = NHL * HD    # local channels = 512
NCC = C // 128   # cin chunks = 8
NTB = T // 128   # token blocks = 16
NPAN = T // 512  # token panels = 4
NQG = T // 512   # query groups of 512 = 4
NHP = NHL // 2   # head pairs = 4

F32 = mybir.dt.float32
F32R = mybir.dt.float32r


def _build_program():
    nc = bacc.Bacc("TRN2", target_bir_lowering=False, debug=False, num_devices=8)

    xT = nc.dram_tensor("xT", [C, T], F32R, kind="ExternalInput").ap()
    wqkT = nc.dram_tensor("wqkT", [C, 2 * CL], F32R, kind="ExternalInput").ap()
    wvT = nc.dram_tensor("wvT", [C, CL], F32R, kind="ExternalInput").ap()
    wprojT = nc.dram_tensor("wprojT", [CL, C], F32R, kind="ExternalInput").ap()
    masks = nc.dram_tensor("masks", [128, 4 * 512], F32R, kind="ExternalInput").ap()
    ones = nc.dram_tensor("ones", [128, NTB * NHL], F32R, kind="ExternalInput").ap()
    yT = nc.dram_tensor("yT", [C, T], F32, kind="ExternalOutput").ap()

    with tile.TileContext(nc) as tc, ExitStack() as ctx:
        persist = ctx.enter_context(tc.tile_pool(name="persist", bufs=1))
        # Q^T/K^T per head pair: [128, T], partitions = 2 heads x 64
        QT = [persist.tile([128, T], F32R, tag=f"qt{p}", name=f"qt{p}") for p in range(NHP)]
        KT = [persist.tile([128, T], F32R, tag=f"kt{p}", name=f"kt{p}") for p in range(NHP)]
        # V with a ones column appended per head: [tk mod 128, tb, head, 65]
        Vg = persist.tile([128, NTB, NHL, HD + 1], F32R, tag="vg")

        # ones column of V_aug (softmax denominator trick), via strided DMA
        nc.sync.dma_start(
            out=Vg.rearrange("p tb h d -> p (tb h) d")[:, :, HD : HD + 1],
            in_=ones.rearrange("p (n o) -> p n o", o=1),
        )

        # ---------------- phase 1: QKV projection ----------------
        with ExitStack() as p1:
            wpool = p1.enter_context(tc.tile_pool(name="p1w", bufs=1))
            xpool = p1.enter_context(tc.tile_pool(name="p1x", bufs=2))
            ps1 = p1.enter_context(tc.tile_pool(name="p1ps", bufs=4, space="PSUM"))

            wqk_sb = wpool.tile([128, NCC, 2 * CL], F32R, tag="wqk")
            wv_sb = wpool.tile([128, NCC, CL], F32R, tag="wv")
            nc.sync.dma_start(out=wqk_sb, in_=wqkT.rearrange("(cc p) n -> p cc n", p=128))
            nc.sync.dma_start(out=wv_sb, in_=wvT.rearrange("(cc p) n -> p cc n", p=128))

            xTr = xT.rearrange("(cc p) t -> p cc t", p=128)
            for pan in range(NPAN):
                xp = xpool.tile([128, NCC, 512], F32R, tag="xp")
                nc.sync.dma_start(out=xp, in_=xTr[:, :, pan * 512 : (pan + 1) * 512])

                # Q (chunks 0-3) and K (chunks 4-7): out [chan 128, tok 512]
                for ch in range(8):
                    ps = ps1.tile([128, 512], F32, tag="ps")
                    for cc in range(NCC):
                        nc.tensor.matmul(
                            ps,
                            wqk_sb[:, cc, ch * 128 : (ch + 1) * 128],
                            xp[:, cc, :],
                            start=(cc == 0),
                            stop=(cc == NCC - 1),
                        )
                    dest = QT[ch] if ch < 4 else KT[ch - 4]
                    nc.vector.tensor_copy(
                        dest[:, pan * 512 : (pan + 1) * 512], ps
                    )
                # V: out [tok 128, chan 512] for the 4 token blocks of the panel
                for tbl in range(4):
                    tb = pan * 4 + tbl
                    ps = ps1.tile([128, 512], F32, tag="ps")
                    for cc in range(NCC):
                        nc.tensor.matmul(
                            ps,
                            xp[:, cc, tbl * 128 : (tbl + 1) * 128],
                            wv_sb[:, cc, :],
                            start=(cc == 0),
                            stop=(cc == NCC - 1),
                        )
                    nc.vector.tensor_copy(
                        Vg[:, tb, :, 0:HD],
                        ps.rearrange("p (h d) -> p h d", h=NHL),
                    )

        # ---------------- phases 2+3 ----------------
        with ExitStack() as p23:
            otpool = p23.enter_context(tc.tile_pool(name="ot", bufs=1))
            OT = [otpool.tile([128, T], F32R, tag=f"ot{p}", name=f"ot{p}") for p in range(NHP)]
            masks_sb = otpool.tile([128, 4 * 512], F32R, tag="masks")
            nc.sync.dma_start(out=masks_sb, in_=masks)
            wp_sb = otpool.tile([128, NHP, C], F32R, tag="wp")
            nc.sync.dma_start(
                out=wp_sb, in_=wprojT.rearrange("(cc p) n -> p cc n", p=128)
            )

            # ---- phase 2: attention ----
            with ExitStack() as p2:
                ppool = p2.enter_context(tc.tile_pool(name="p2p", bufs=8))
                psS = p2.enter_context(tc.tile_pool(name="p2s", bufs=6, space="PSUM"))
                psO = p2.enter_context(tc.tile_pool(name="p2o", bufs=2, space="PSUM"))

                for hp in range(NHP):
                    for qg in range(NQG):
                        nkb = 4 * qg + 4  # causal extent in k-blocks
                        OA = psO.tile([HD + 1, 512], F32, tag="o")
                        OB = psO.tile([HD + 1, 512], F32, tag="o")
                        qsl = slice(qg * 512, (qg + 1) * 512)
                        for jk in range(nkb):
                            ksl = slice(jk * 128, (jk + 1) * 128)
                            SA = psS.tile([128, 512], F32, tag="s")
                            SB = psS.tile([128, 512], F32, tag="s")
                            nc.tensor.matmul(
                                SA, KT[hp][0:64, ksl], QT[hp][0:64, qsl],
                                start=True, stop=True,
                            )
                            nc.tensor.matmul(
                                SB, KT[hp][64:128, ksl], QT[hp][64:128, qsl],
                                start=True, stop=True,
                            )
                            PA = ppool.tile([128, 512], F32R, tag="pt")
                            PB = ppool.tile([128, 512], F32R, tag="pt")
                            nc.scalar.activation(PA, SA, mybir.ActivationFunctionType.Exp)
                            nc.scalar.activation(PB, SB, mybir.ActivationFunctionType.Exp)
                            if jk >= 4 * qg:  # diagonal-intersecting block
                                r = jk - 4 * qg
                                msl = slice(r * 512, (r + 1) * 512)
                                nc.vector.tensor_mul(PA, PA, masks_sb[:, msl])
                                nc.vector.tensor_mul(PB, PB, masks_sb[:, msl])
                            nc.tensor.matmul(
                                OA, Vg[:, jk, 2 * hp, :], PA,
                                start=(jk == 0), stop=(jk == nkb - 1),
                            )
                            nc.tensor.matmul(
                                OB, Vg[:, jk, 2 * hp + 1, :], PB,
                                start=(jk == 0), stop=(jk == nkb - 1),
                            )
                        # normalize: 1/denominator, broadcast over 64 partitions,
                        # multiply while copying O^T out of PSUM
                        for h01, Oacc in ((0, OA), (1, OB)):
                            drow = ppool.tile([1, 512], F32, tag="drow", bufs=2)
                            nc.vector.tensor_copy(drow, Oacc[HD : HD + 1, :])
                            rcp = ppool.tile([1, 512], F32, tag="rcp", bufs=2)
                            nc.vector.reciprocal_approx_fast(rcp, drow)
                            bc = ppool.tile([64, 512], F32, tag="bc", bufs=2)
                            nc.gpsimd.partition_broadcast(bc, rcp)
                            nc.vector.tensor_mul(
                                OT[hp][h01 * 64 : (h01 + 1) * 64, qsl],
                                Oacc[0:HD, :],
                                bc,
                            )

            # ---- phase 3: output projection (partial, local channels) ----
            with ExitStack() as p3:
                o3pool = p3.enter_context(tc.tile_pool(name="p3o", bufs=4))
                ps3 = p3.enter_context(tc.tile_pool(name="p3ps", bufs=4, space="PSUM"))
                for ch in range(C // 128):
                    for tg in range(NPAN):
                        ps = ps3.tile([128, 512], F32, tag="ps")
                        for cc in range(NHP):
                            nc.tensor.matmul(
                                ps,
                                wp_sb[:, cc, ch * 128 : (ch + 1) * 128],
                                OT[cc][:, tg * 512 : (tg + 1) * 512],
                                start=(cc == 0),
                                stop=(cc == NHP - 1),
                            )
                        o = o3pool.tile([128, 512], F32, tag="o3")
                        nc.vector.tensor_copy(o, ps)
                        nc.sync.dma_start(
                            out=yT[ch * 128 : (ch + 1) * 128, tg * 512 : (tg + 1) * 512],
                            in_=o,
                        )

    nc.compile()
    return nc


_NC = None


def _get_program():
    global _NC
    if _NC is None:
        _NC = _build_program()
    return _NC


def _make_masks() -> np.ndarray:
    # mask_r[tk, tq] = 1 if tq >= 128*r + tk else 0, r = 0..3, laid side by side
    tk = np.arange(128)[:, None]
    tq = np.arange(512)[None, :]
    return np.concatenate(
        [(tq >= 128 * r + tk).astype(np.float32) for r in range(4)], axis=1
    )


def make_in_maps(x, w_qkv, w_proj):
    x = np.asarray(x, dtype=np.float32)
    w_qkv = np.asarray(w_qkv, dtype=np.float32)
    w_proj = np.asarray(w_proj, dtype=np.float32)
    masks = _make_masks()
    scale = np.float32(HD ** -0.5)
    in_maps = []
    for i in range(8):
        b, hh = i // 2, i % 2
        ch0 = hh * CL
        wq = w_qkv[ch0 : ch0 + CL, :] * scale
        wk = w_qkv[C + ch0 : C + ch0 + CL, :]
        wv = w_qkv[2 * C + ch0 : 2 * C + ch0 + CL, :]
        in_maps.append(
            {
                "xT": np.ascontiguousarray(x[b].T),
                "wqkT": np.ascontiguousarray(np.concatenate([wq, wk], 0).T),
                "wvT": np.ascontiguousarray(wv.T),
                "wprojT": np.ascontiguousarray(w_proj[:, ch0 : ch0 + CL].T),
                "masks": masks,
                "ones": np.ones((128, NTB * NHL), dtype=np.float32),
            }
        )
    return in_maps


def assemble_output(results, b_proj):
    b_proj = np.asarray(b_proj, dtype=np.float32)
    y = np.empty((B, T, C), dtype=np.float32)
    for b in range(B):
        y[b] = (results[2 * b]["yT"] + results[2 * b + 1]["yT"]).T + b_proj
    return y


def kernel(x, w_qkv, w_proj, b_proj):
    nc = _get_program()
    in_maps = make_in_maps(x, w_qkv, w_proj)
    res = run_bass_kernel_spmd(nc, in_maps, core_ids=list(range(8)))
    return assemble_output(res.results, b_proj)


if __name__ == "__main__":
    rng = np.random.default_rng(0)
    x = rng.standard_normal((B, T, C), dtype=np.float32)
    w_qkv = (rng.standard_normal((3 * C, C), dtype=np.float32) * C ** -0.5).astype(
        np.float32
    )
    w_proj = (rng.standard_normal((C, C), dtype=np.float32) * C ** -0.5).astype(
        np.float32
    )
    b_proj = np.zeros(C, dtype=np.float32)
    y = kernel(x, w_qkv, w_proj, b_proj)
    print("out", y.shape, y.dtype, float(np.abs(y).max()))
